# revision 19
# baseline (speedup 1.0000x reference)
"""DNN_Beamformer (MVDR + attention reference) on 8 Trainium2 NeuronCores.

B-sharded: one batch element per core, full inputs in / full output out.
Two Bass kernel launches per call:
  K1: masked cross-channel PSDs for both masks on the PE array with T on the
      contraction axis (data stays in its natural t-major layout); mask
      channel-sums on DVE, PE-transposed to t-major bf16; the mask-weighted
      data (xw) is built in bf16 pairs at DVE 2x rate.  Also emits a bf16
      copy of the data for K2.  Normalization is folded into host scalars.
  host middle: PSD normalization, attention reference (MLP + softmax),
      8x8 complex inverse, MVDR weights -> beamforming vector bf (tiny).
  K2: beamforming apply enh = conj(bf)^T x via PE block-diagonal rotation
      matmuls on DMA-xbar-transposed bf16 tiles; output lands t-major.

When BEAM_TRACE is set, each kernel runs twice: once traced (timing only -
NTFF profiling corrupts concurrently running cores) and once clean for the
actual outputs.

If anything in the device path fails, falls back to a pure-numpy pipeline
so the output is always correct.
"""

import os
import sys

import numpy as np

sys.path.insert(0, '/opt/trn_rl_repo')

B, T, C, F = 8, 1024, 8, 513
A = 320
EPS_MASK = 1e-6
EPS_PSD = 1e-15
EPS_MVDR = 1e-15
SCALING = 2.0

P = 128
NT = T // P                      # 8 t-tiles
F_TILES = [(i * P, min(P, F - i * P)) for i in range((F + P - 1) // P)]  # 4x128 + 1
NG = (F + 3) // 4                # 129 col-packed groups of 4 freqs
PSUM_W = 16                      # one (32,16) psd slot per (f)
BANK = 512                       # PSUM bank free f32


def _off(g):
    return (g // 32) * BANK + (g % 32) * PSUM_W


def _build_psd_kernel():
    import concourse.bacc as bacc
    import concourse.mybir as mybir
    from concourse.tile import TileContext
    from concourse import masks as cmasks

    nc = bacc.Bacc(None, target_bir_lowering=False)
    fp = mybir.dt.float32
    bh = mybir.dt.bfloat16
    data_d = nc.dram_tensor("data", [T, C, F, 2], fp, kind="ExternalInput")
    ms_d = nc.dram_tensor("ms", [F, C, T], fp, kind="ExternalInput")
    mn_d = nc.dram_tensor("mn", [F, C, T], fp, kind="ExternalInput")
    psd_d = nc.dram_tensor("psd_raw", [P, 5 * BANK], fp, kind="ExternalOutput")
    msum_d = nc.dram_tensor("msum", [P, 10], fp, kind="ExternalOutput")
    dbf_d = nc.dram_tensor("data_bf", [T, C, F, 2], bh, kind="ExternalOutput")

    mult = mybir.AluOpType.mult
    add = mybir.AluOpType.add

    with TileContext(nc) as tc:
        with tc.tile_pool(name="const", bufs=1) as cpool, \
             tc.tile_pool(name="big", bufs=2) as bigpool, \
             tc.tile_pool(name="bfc", bufs=2) as bfpool, \
             tc.tile_pool(name="mmean", bufs=2) as mmpool, \
             tc.tile_pool(name="mt", bufs=1) as mtpool, \
             tc.tile_pool(name="tp", bufs=2, space="PSUM") as tppool, \
             tc.tile_pool(name="xw", bufs=2) as xwpool, \
             tc.tile_pool(name="psum", bufs=1, space="PSUM") as pspool, \
             tc.tile_pool(name="out", bufs=1) as opool:

            ident = cpool.tile([P, P], fp)
            cmasks.make_identity(nc, ident[:])

            msum_t = opool.tile([P, 10], fp, tag="msum")
            nc.vector.memset(msum_t[:], 0.0)

            # --- phase A: mask channel sums -> mtd (t-major, bf16, ri-dup) ---
            mtds = [[mtpool.tile([P, F, 2], bh, tag=f"mtd{mk}_{it}",
                                 name=f"mtd{mk}_{it}")
                     for it in range(NT)] for mk in range(2)]
            for mk, md in enumerate((ms_d, mn_d)):
                for ft, (f0, pf) in enumerate(F_TILES):
                    mtile = bigpool.tile([P, C, T], fp, tag="big", name="mtile")
                    nc.sync.dma_start(mtile[:pf], md[f0:f0 + pf])
                    m = mmpool.tile([P, T], fp, tag="m")
                    nc.vector.tensor_tensor(mtile[:pf, 0], mtile[:pf, 0],
                                            mtile[:pf, 1], add)
                    nc.vector.tensor_tensor(mtile[:pf, 2], mtile[:pf, 2],
                                            mtile[:pf, 3], add)
                    nc.vector.tensor_tensor(mtile[:pf, 4], mtile[:pf, 4],
                                            mtile[:pf, 5], add)
                    nc.vector.tensor_tensor(mtile[:pf, 6], mtile[:pf, 6],
                                            mtile[:pf, 7], add)
                    nc.vector.tensor_tensor(mtile[:pf, 0], mtile[:pf, 0],
                                            mtile[:pf, 2], add)
                    nc.vector.tensor_tensor(mtile[:pf, 4], mtile[:pf, 4],
                                            mtile[:pf, 6], add)
                    nc.vector.tensor_tensor(m[:pf], mtile[:pf, 0],
                                            mtile[:pf, 4], add)
                    nc.vector.tensor_reduce(
                        msum_t[0:pf, mk * 5 + ft:mk * 5 + ft + 1], m[:pf],
                        axis=mybir.AxisListType.X, op=add)
                    # transpose (pf, T) -> t-major; duplicate along ri in bf16
                    for it in range(NT):
                        tp = tppool.tile([P, P], fp, tag="tp")
                        nc.tensor.transpose(tp[:, :pf],
                                            m[:pf, it * P:(it + 1) * P],
                                            ident[:pf, :pf])
                        nc.scalar.copy(mtds[mk][it][:, f0:f0 + pf, 0],
                                       tp[:, :pf])
                        nc.scalar.copy(mtds[mk][it][:, f0:f0 + pf, 1],
                                       tp[:, :pf])
            nc.sync.dma_start(msum_d[:], msum_t[:])

            # --- phase B: bf16 cast + xw + PSD matmuls (PSUM-accumulated) ---
            # xw rows ordered (mk, c, ri); per f: out(32,16) with K=128t,
            # 4 freqs col-packed across the PE array.
            psd_ps = pspool.tile([P, 5 * BANK], fp, tag="psd")
            FC = 171  # f-chunk; 513 = 3*171
            for it in range(NT):
                dt = bigpool.tile([P, C, F, 2], fp, tag="big", name="dt")
                nc.sync.dma_start(dt[:], data_d[it * P:(it + 1) * P])
                bfc = bfpool.tile([P, C, F, 2], bh, tag="bfc", name="bfc")
                nc.scalar.copy(bfc[:], dt[:])
                nc.sync.dma_start(dbf_d[it * P:(it + 1) * P], bfc[:])
                for fc0 in range(0, F, FC):
                    fw = min(FC, F - fc0)
                    xw = xwpool.tile([P, FC, 32], bh, tag="xw")
                    for mk in range(2):
                        for c in range(C):
                            r0 = mk * 16 + 2 * c
                            nc.vector.tensor_tensor(
                                xw[:, :fw, r0:r0 + 2],
                                bfc[:, c, fc0:fc0 + fw, :],
                                mtds[mk][it][:, fc0:fc0 + fw, :], mult)
                    for fl in range(fw):
                        f = fc0 + fl
                        g, j = f // 4, f % 4
                        o = _off(g)
                        nc.tensor.matmul(
                            psd_ps[32 * j:32 * (j + 1), o:o + PSUM_W],
                            xw[:, fl, :],
                            bfc[:, :, f, :],
                            # start=True clears has_written for the WHOLE
                            # bank -> only the first matmul per bank sets it
                            start=(it == 0 and f % 128 == 0),
                            stop=(it == NT - 1),
                            skip_group_check=True,
                            tile_position=(0, 32 * j))

            psd_sb = opool.tile([P, 5 * BANK], fp, tag="psdsb")
            nc.scalar.copy(psd_sb[:], psd_ps[:])
            nc.sync.dma_start(psd_d[:], psd_sb[:])
    nc.compile()
    return nc


FB = 9  # f-blocks of 64 for the PE apply (last block holds only f=512)


def _build_apply_kernel_v2():
    """Apply via PE block-diag matmuls on DMA-xbar-transposed bf16 data.

    Per (t-tile, channel): one DMA transpose loads ((fl,ri), fb, t) for all
    8 f-blocks.  Per f-block: 8 matmuls (one per channel) with the 2x2
    block-diagonal rotation weights accumulate in PSUM; out lands t-major.
    f=512 handled via a tiny PE-transpose + K=16 matmul.  Transposes and
    output DMAs alternate between the two HWDGE engines.
    """
    import concourse.bacc as bacc
    import concourse.mybir as mybir
    from concourse.tile import TileContext
    from concourse import masks as cmasks

    nc = bacc.Bacc(None, target_bir_lowering=False)
    fp = mybir.dt.float32
    bh = mybir.dt.bfloat16
    dbf_d = nc.dram_tensor("data_bf", [T, C, F, 2], bh, kind="ExternalInput")
    wt_d = nc.dram_tensor("wt", [P, FB, C, P], bh, kind="ExternalInput")
    out_d = nc.dram_tensor("out", [T, F, 2], fp, kind="ExternalOutput")

    with TileContext(nc) as tc:
        with tc.tile_pool(name="const", bufs=1) as cpool, \
             tc.tile_pool(name="xt", bufs=2) as xpool, \
             tc.tile_pool(name="d512", bufs=2) as dpool, \
             tc.tile_pool(name="tp", bufs=2, space="PSUM") as tppool, \
             tc.tile_pool(name="op", bufs=3, space="PSUM") as oppool, \
             tc.tile_pool(name="oc", bufs=2, space="PSUM") as ocpool, \
             tc.tile_pool(name="eo", bufs=2) as epool:
            identb = cpool.tile([P, P], bh)
            cmasks.make_identity(nc, identb[:])
            wt = cpool.tile([P, FB, C, P], bh, tag="wt")
            nc.sync.dma_start(wt[:], wt_d[:])

            # all f=512 columns upfront (one DMA, overlaps everything)
            d512a = dpool.tile([P, NT, C, 2], bh, tag="d512", name="d512a")
            for it2 in range(NT):
                nc.sync.dma_start(d512a[:, it2, :, :],
                                  dbf_d[it2 * P:(it2 + 1) * P, :, 512, :])
            for it in range(NT):
                t0 = it * P
                xts = [xpool.tile([P, 8, P], bh, tag=f"xt{c}",
                                  name=f"xt{c}_{it}") for c in range(C)]
                for c in range(C):
                    nc.sync.dma_start(xts[c][:],
                                      dbf_d[t0:t0 + P, c, 0:512, :],
                                      transpose=True)
                eo = epool.tile([P, F, 2], fp, tag="eo", name="eo2")
                # f=512 transpose FIRST: its DVE copy overlaps the fb matmul
                # stream so the PE never stalls on the serial f=512 chain
                tp = tppool.tile([P, P], bh, tag="tp", name="tp512")
                nc.tensor.transpose(tp[:16, :], d512a[:, it, :, :], identb[:])
                x512 = dpool.tile([16, P], bh, tag="x512", name="x512")
                nc.vector.tensor_copy(x512[:], tp[:16, :])
                for fb in range(8):
                    op = oppool.tile([P, P], fp, tag="op", name="op2")
                    for c in range(C):
                        nc.tensor.matmul(op[:], xts[c][:, fb, :],
                                         wt[:, fb, c, :],
                                         start=(c == 0), stop=(c == C - 1))
                    f0 = 64 * fb
                    if fb % 2 == 0:
                        nc.vector.tensor_copy(eo[:, f0:f0 + 64, :], op[:])
                    else:
                        nc.scalar.copy(eo[:, f0:f0 + 64, :], op[:])
                oc = ocpool.tile([P, 2], fp, tag="oc", name="oc2")
                nc.tensor.matmul(oc[:], x512[:], wt[:16, 8, 0, :2],
                                 start=True, stop=True)
                nc.vector.tensor_copy(eo[:, 512, :], oc[:])
                nc.sync.dma_start(out_d[t0:t0 + P], eo[:])
    nc.compile()
    return nc


def _build_wt(bf):
    """bf: (F, C) complex64 -> Wt (128, FB, C, 128) bf16 (block-diag rot).

    fb<8: W[fb, c, (fl,ri), (fl2,ri2)] block-diagonal per fl.
    fb=8 slot [0:16, 8, 0, 0:2]: rows (c,ri), cols ri2 for f=512.
    """
    import ml_dtypes
    wa = bf.real.astype(np.float32)   # (F, C)
    wb = bf.imag.astype(np.float32)
    W = np.zeros((FB, C, P, P), np.float32)
    fl = np.arange(64)
    for fb in range(8):
        f = 64 * fb + fl
        for c in range(C):
            W[fb, c, 2 * fl, 2 * fl] = wa[f, c]
            W[fb, c, 2 * fl + 1, 2 * fl] = wb[f, c]
            W[fb, c, 2 * fl, 2 * fl + 1] = -wb[f, c]
            W[fb, c, 2 * fl + 1, 2 * fl + 1] = wa[f, c]
    for c in range(C):
        W[8, 0, 2 * c, 0] = wa[512, c]
        W[8, 0, 2 * c + 1, 0] = wb[512, c]
        W[8, 0, 2 * c, 1] = -wb[512, c]
        W[8, 0, 2 * c + 1, 1] = wa[512, c]
    return np.ascontiguousarray(
        W.transpose(2, 0, 1, 3)).astype(ml_dtypes.bfloat16)


def _decode_psd(raw, msum):
    """raw: (B,128,2560), msum: (B,128,10) -> psd_s, psd_n (B,F,C,C) c64."""
    nb = raw.shape[0]
    slots = np.empty((nb, 32, F, 16), np.float32)
    g = np.arange(F) // 4
    j = np.arange(F) % 4
    off = (g // 32) * BANK + (g % 32) * PSUM_W
    for jj in range(4):
        sel = j == jj
        cols = off[sel][:, None] + np.arange(16)[None]
        slots[:, :, sel, :] = raw[:, 32 * jj:32 * (jj + 1), :][
            :, :, cols.reshape(-1)].reshape(nb, 32, sel.sum(), 16)
    ms_sum = np.empty((nb, 2, F), np.float32)
    for mk in range(2):
        for ft, (f0, pf) in enumerate(F_TILES):
            ms_sum[:, mk, f0:f0 + pf] = msum[:, :pf, mk * 5 + ft]
    psds = []
    for mk in range(2):
        r = slots[:, 16 * mk:16 * mk + 16:2]     # (B, 8c, F, 16)
        i = slots[:, 16 * mk + 1:16 * mk + 16:2]
        re = r[..., 0::2] + i[..., 1::2]          # (B, c, F, e)
        im = i[..., 0::2] - r[..., 1::2]
        psd = (re + 1j * im).astype(np.complex64).transpose(0, 2, 1, 3)  # (B,F,c,e)
        scale = 1.0 / (ms_sum[:, mk] + C * EPS_PSD)
        psds.append(psd * scale[:, :, None, None].astype(np.complex64))
    return psds[0], psds[1]


def _middle(psd_s, psd_n, W_psd, b_psd, w_gvec, b_gvec):
    eye = np.eye(C, dtype=bool)
    psd = np.where(eye[None, None], np.complex64(0), psd_s)
    psd = np.swapaxes(psd.sum(axis=-1) / (C - 1), -1, -2)
    psd_feat = np.abs(psd).astype(np.float32)
    e = np.tanh(psd_feat @ W_psd + b_psd) @ w_gvec + b_gvec[0]
    e = SCALING * e
    e = e - e.max(axis=-1, keepdims=True)
    ex = np.exp(e)
    u = (ex / ex.sum(axis=-1, keepdims=True)).astype(np.float32)
    psd_n_reg = psd_n + (EPS_MVDR * np.eye(C)).astype(np.complex64)
    num = np.matmul(np.linalg.inv(psd_n_reg), psd_s)
    trace = np.einsum('bfcc->bf', num)
    ws = num / (trace[..., None, None] + EPS_MVDR)
    bf = np.einsum('bfec,bc->bfe', ws, u.astype(ws.dtype))  # (B,F,C)
    return bf


_RES = {"t_psd": None, "t_apply": None}


def _run(nc, in_maps, trace=False):
    from concourse.bass_utils import run_bass_kernel_spmd
    return run_bass_kernel_spmd(nc, in_maps, core_ids=list(range(B)),
                                trace=trace)


def _device_pipeline(data_ri, mask_speech, mask_noise,
                     W_psd, b_psd, w_gvec, b_gvec):
    # NTFF profiling corrupts execution on the non-profiled cores AND
    # poisons subsequent executions in the same process, so compute ALL
    # real outputs with clean runs first, then (if tracing was requested)
    # re-run both kernels traced purely for the timing numbers.
    trace = bool(os.environ.get("BEAM_TRACE"))
    nc1 = _build_psd_kernel()
    in1 = [{"data": data_ri[b],
            "ms": np.ascontiguousarray(mask_speech[b], np.float32),
            "mn": np.ascontiguousarray(mask_noise[b], np.float32)}
           for b in range(B)]
    r1 = _run(nc1, in1)
    raw = np.stack([r["psd_raw"] for r in r1.results])
    msum = np.stack([r["msum"] for r in r1.results])
    dbf = np.stack([r["data_bf"] for r in r1.results])

    psd_s, psd_n = _decode_psd(raw, msum)
    bf = _middle(psd_s, psd_n, W_psd, b_psd, w_gvec, b_gvec)

    nc2 = _build_apply_kernel_v2()
    in2 = [{"data_bf": dbf[b], "wt": _build_wt(bf[b])} for b in range(B)]
    r2 = _run(nc2, in2)
    out = np.stack([r["out"] for r in r2.results])

    if trace:
        _RES["t_psd"] = _run(nc1, in1, trace=True).exec_time_ns
        _RES["t_apply"] = _run(nc2, in2, trace=True).exec_time_ns
    return out


def _numpy_pipeline(data_ri, mask_speech, mask_noise,
                    W_psd, b_psd, w_gvec, b_gvec):
    data = data_ri[..., 0] + 1j * data_ri[..., 1]
    x = np.ascontiguousarray(np.transpose(data, (0, 3, 2, 1)))  # (B,F,C,T)
    psds = []
    for mask in (mask_speech, mask_noise):
        m = np.clip(mask, EPS_MASK, None).mean(axis=-2)
        m = m / (m.sum(axis=-1, keepdims=True) + EPS_PSD)
        xw = x * m[:, :, None, :].astype(x.dtype)
        psds.append(np.matmul(xw, np.conj(np.swapaxes(x, -1, -2))))
    bf = _middle(psds[0], psds[1], W_psd, b_psd, w_gvec, b_gvec)
    enh = np.einsum('bfc,bfct->bft', np.conj(bf), x)
    enh = np.swapaxes(enh, -1, -2)
    return np.stack([enh.real, enh.imag], axis=-1).astype(np.float32)


def kernel(data_ri, mask_speech, mask_noise, W_psd, b_psd, w_gvec, b_gvec,
           ilens):
    data_ri = np.ascontiguousarray(data_ri, dtype=np.float32)
    mask_speech = np.asarray(mask_speech, np.float32)
    mask_noise = np.asarray(mask_noise, np.float32)
    W_psd = np.asarray(W_psd, np.float32)
    b_psd = np.asarray(b_psd, np.float32)
    w_gvec = np.asarray(w_gvec, np.float32)
    b_gvec = np.asarray(b_gvec, np.float32)
    if os.environ.get("BEAM_NO_DEVICE"):
        return _numpy_pipeline(data_ri, mask_speech, mask_noise,
                               W_psd, b_psd, w_gvec, b_gvec)
    import signal
    old = None
    try:
        if hasattr(signal, "SIGALRM"):
            def _timeout(signum, frame):
                raise TimeoutError("device pipeline watchdog")
            old = signal.signal(signal.SIGALRM, _timeout)
            signal.alarm(900)
        return _device_pipeline(data_ri, mask_speech, mask_noise,
                                W_psd, b_psd, w_gvec, b_gvec)
    except Exception as exc:  # device unavailable -> still return correctly
        sys.stderr.write(f"device pipeline failed ({exc!r}); numpy fallback\n")
        return _numpy_pipeline(data_ri, mask_speech, mask_noise,
                               W_psd, b_psd, w_gvec, b_gvec)
    finally:
        if old is not None:
            signal.alarm(0)
            signal.signal(signal.SIGALRM, old)


# revision 20
# speedup vs baseline: 1.0497x; 1.0497x over previous
"""DNN_Beamformer (MVDR + attention reference) on 8 Trainium2 NeuronCores.

B-sharded: one batch element per core, full inputs in / full output out.
Two Bass kernel launches per call:
  K1: masked cross-channel PSDs for both masks on the PE array with T on the
      contraction axis (data stays in its natural t-major layout); mask
      channel-sums on DVE, PE-transposed to t-major bf16; the mask-weighted
      data (xw) is built in bf16 pairs at DVE 2x rate.  Also emits a bf16
      copy of the data for K2.  Normalization is folded into host scalars.
  host middle: PSD normalization, attention reference (MLP + softmax),
      8x8 complex inverse, MVDR weights -> beamforming vector bf (tiny).
  K2: beamforming apply enh = conj(bf)^T x via PE block-diagonal rotation
      matmuls on DMA-xbar-transposed bf16 tiles; output lands t-major.

When BEAM_TRACE is set, each kernel runs twice: once traced (timing only -
NTFF profiling corrupts concurrently running cores) and once clean for the
actual outputs.

If anything in the device path fails, falls back to a pure-numpy pipeline
so the output is always correct.
"""

import os
import sys

import numpy as np

sys.path.insert(0, '/opt/trn_rl_repo')

B, T, C, F = 8, 1024, 8, 513
A = 320
EPS_MASK = 1e-6
EPS_PSD = 1e-15
EPS_MVDR = 1e-15
SCALING = 2.0

P = 128
NT = T // P                      # 8 t-tiles
F_TILES = [(i * P, min(P, F - i * P)) for i in range((F + P - 1) // P)]  # 4x128 + 1
NG = (F + 3) // 4                # 129 col-packed groups of 4 freqs
PSUM_W = 16                      # one (32,16) psd slot per (f)
BANK = 512                       # PSUM bank free f32


def _off(g):
    return (g // 32) * BANK + (g % 32) * PSUM_W


def _build_psd_kernel():
    import concourse.bacc as bacc
    import concourse.mybir as mybir
    from concourse.tile import TileContext
    from concourse import masks as cmasks

    nc = bacc.Bacc(None, target_bir_lowering=False)
    fp = mybir.dt.float32
    bh = mybir.dt.bfloat16
    data_d = nc.dram_tensor("data", [T, C, F, 2], fp, kind="ExternalInput")
    ms_d = nc.dram_tensor("ms", [F, C, T], fp, kind="ExternalInput")
    mn_d = nc.dram_tensor("mn", [F, C, T], fp, kind="ExternalInput")
    psd_d = nc.dram_tensor("psd_raw", [P, 5 * BANK], fp, kind="ExternalOutput")
    msum_d = nc.dram_tensor("msum", [P, 10], fp, kind="ExternalOutput")
    dbf_d = nc.dram_tensor("data_bf", [T, C, F, 2], bh, kind="ExternalOutput")

    mult = mybir.AluOpType.mult
    add = mybir.AluOpType.add

    with TileContext(nc) as tc:
        with tc.tile_pool(name="const", bufs=1) as cpool, \
             tc.tile_pool(name="big", bufs=2) as bigpool, \
             tc.tile_pool(name="bfc", bufs=2) as bfpool, \
             tc.tile_pool(name="mmean", bufs=2) as mmpool, \
             tc.tile_pool(name="mt", bufs=1) as mtpool, \
             tc.tile_pool(name="tp", bufs=2, space="PSUM") as tppool, \
             tc.tile_pool(name="xw", bufs=2) as xwpool, \
             tc.tile_pool(name="psum", bufs=1, space="PSUM") as pspool, \
             tc.tile_pool(name="out", bufs=1) as opool:

            ident = cpool.tile([P, P], fp)
            cmasks.make_identity(nc, ident[:])

            msum_t = opool.tile([P, 10], fp, tag="msum")
            nc.vector.memset(msum_t[:], 0.0)

            # --- phase A: mask channel sums -> mtd (t-major, bf16, ri-dup) ---
            mtds = [[mtpool.tile([P, F, 2], bh, tag=f"mtd{mk}_{it}",
                                 name=f"mtd{mk}_{it}")
                     for it in range(NT)] for mk in range(2)]
            for mk, md in enumerate((ms_d, mn_d)):
                for ft, (f0, pf) in enumerate(F_TILES):
                    mtile = bigpool.tile([P, C, T], fp, tag="big", name="mtile")
                    nc.sync.dma_start(mtile[:pf], md[f0:f0 + pf])
                    m = mmpool.tile([P, T], fp, tag="m")
                    nc.vector.tensor_tensor(mtile[:pf, 0], mtile[:pf, 0],
                                            mtile[:pf, 1], add)
                    nc.vector.tensor_tensor(mtile[:pf, 2], mtile[:pf, 2],
                                            mtile[:pf, 3], add)
                    nc.vector.tensor_tensor(mtile[:pf, 4], mtile[:pf, 4],
                                            mtile[:pf, 5], add)
                    nc.vector.tensor_tensor(mtile[:pf, 6], mtile[:pf, 6],
                                            mtile[:pf, 7], add)
                    nc.vector.tensor_tensor(mtile[:pf, 0], mtile[:pf, 0],
                                            mtile[:pf, 2], add)
                    nc.vector.tensor_tensor(mtile[:pf, 4], mtile[:pf, 4],
                                            mtile[:pf, 6], add)
                    nc.vector.tensor_tensor(m[:pf], mtile[:pf, 0],
                                            mtile[:pf, 4], add)
                    nc.vector.tensor_reduce(
                        msum_t[0:pf, mk * 5 + ft:mk * 5 + ft + 1], m[:pf],
                        axis=mybir.AxisListType.X, op=add)
                    # transpose (pf, T) -> t-major; duplicate along ri in bf16
                    for it in range(NT):
                        tp = tppool.tile([P, P], fp, tag="tp")
                        nc.tensor.transpose(tp[:, :pf],
                                            m[:pf, it * P:(it + 1) * P],
                                            ident[:pf, :pf])
                        nc.scalar.copy(mtds[mk][it][:, f0:f0 + pf, 0],
                                       tp[:, :pf])
                        nc.scalar.copy(mtds[mk][it][:, f0:f0 + pf, 1],
                                       tp[:, :pf])
            nc.sync.dma_start(msum_d[:], msum_t[:])

            # --- phase B: bf16 cast + xw + PSD matmuls (PSUM-accumulated) ---
            # xw rows ordered (mk, c, ri); per f: out(32,16) with K=128t,
            # 4 freqs col-packed across the PE array.
            psd_ps = pspool.tile([P, 5 * BANK], fp, tag="psd")
            FC = 171  # f-chunk; 513 = 3*171
            for it in range(NT):
                dt = bigpool.tile([P, C, F, 2], fp, tag="big", name="dt")
                nc.sync.dma_start(dt[:], data_d[it * P:(it + 1) * P])
                bfc = bfpool.tile([P, C, F, 2], bh, tag="bfc", name="bfc")
                nc.scalar.copy(bfc[:], dt[:])
                nc.sync.dma_start(dbf_d[it * P:(it + 1) * P], bfc[:])
                for fc0 in range(0, F, FC):
                    fw = min(FC, F - fc0)
                    xw = xwpool.tile([P, FC, 32], bh, tag="xw")
                    for mk in range(2):
                        for c in range(C):
                            r0 = mk * 16 + 2 * c
                            nc.vector.tensor_tensor(
                                xw[:, :fw, r0:r0 + 2],
                                bfc[:, c, fc0:fc0 + fw, :],
                                mtds[mk][it][:, fc0:fc0 + fw, :], mult)
                    for fl in range(fw):
                        f = fc0 + fl
                        g, j = f // 4, f % 4
                        o = _off(g)
                        nc.tensor.matmul(
                            psd_ps[32 * j:32 * (j + 1), o:o + PSUM_W],
                            xw[:, fl, :],
                            bfc[:, :, f, :],
                            # start=True clears has_written for the WHOLE
                            # bank -> only the first matmul per bank sets it
                            start=(it == 0 and f % 128 == 0),
                            stop=(it == NT - 1),
                            skip_group_check=True,
                            tile_position=(0, 32 * j))

            psd_sb = opool.tile([P, 5 * BANK], fp, tag="psdsb")
            nc.scalar.copy(psd_sb[:], psd_ps[:])
            nc.sync.dma_start(psd_d[:], psd_sb[:])
    nc.compile()
    return nc


FB = 9  # f-blocks of 64 for the PE apply (last block holds only f=512)


def _build_apply_kernel_v2():
    """Apply via PE block-diag matmuls on DMA-xbar-transposed bf16 data.

    Per (t-tile, channel): one DMA transpose loads ((fl,ri), fb, t) for all
    8 f-blocks.  Per f-block: 8 matmuls (one per channel) with the 2x2
    block-diagonal rotation weights accumulate in PSUM; out lands t-major.
    f=512 handled via a tiny PE-transpose + K=16 matmul.  Transposes and
    output DMAs alternate between the two HWDGE engines.
    """
    import concourse.bacc as bacc
    import concourse.mybir as mybir
    from concourse.tile import TileContext
    from concourse import masks as cmasks

    nc = bacc.Bacc(None, target_bir_lowering=False)
    fp = mybir.dt.float32
    bh = mybir.dt.bfloat16
    dbf_d = nc.dram_tensor("data_bf", [T, C, F, 2], bh, kind="ExternalInput")
    wt_d = nc.dram_tensor("wt", [P, FB, C, P], bh, kind="ExternalInput")
    out_d = nc.dram_tensor("out", [T, F, 2], fp, kind="ExternalOutput")

    with TileContext(nc) as tc:
        with tc.tile_pool(name="const", bufs=1) as cpool, \
             tc.tile_pool(name="xt", bufs=2) as xpool, \
             tc.tile_pool(name="d512", bufs=2) as dpool, \
             tc.tile_pool(name="tp", bufs=2, space="PSUM") as tppool, \
             tc.tile_pool(name="op", bufs=3, space="PSUM") as oppool, \
             tc.tile_pool(name="oc", bufs=2, space="PSUM") as ocpool, \
             tc.tile_pool(name="eo", bufs=2) as epool:
            identb = cpool.tile([P, P], bh)
            cmasks.make_identity(nc, identb[:])
            wt = cpool.tile([P, FB, C, P], bh, tag="wt")
            nc.scalar.dma_start(wt[:], wt_d[:])

            # all f=512 columns upfront (one DMA, overlaps everything)
            d512a = dpool.tile([P, NT, C, 2], bh, tag="d512", name="d512a")
            for it2 in range(NT):
                nc.scalar.dma_start(d512a[:, it2, :, :],
                                    dbf_d[it2 * P:(it2 + 1) * P, :, 512, :])
            for it in range(NT):
                t0 = it * P
                xts = [xpool.tile([P, 8, P], bh, tag=f"xt{c}",
                                  name=f"xt{c}_{it}") for c in range(C)]
                for c in range(C):
                    nc.sync.dma_start(xts[c][:],
                                      dbf_d[t0:t0 + P, c, 0:512, :],
                                      transpose=True)
                eo = epool.tile([P, F, 2], fp, tag="eo", name="eo2")
                # f=512 transpose FIRST: its DVE copy overlaps the fb matmul
                # stream so the PE never stalls on the serial f=512 chain
                tp = tppool.tile([P, P], bh, tag="tp", name="tp512")
                nc.tensor.transpose(tp[:16, :], d512a[:, it, :, :], identb[:])
                x512 = dpool.tile([16, P], bh, tag="x512", name="x512")
                nc.vector.tensor_copy(x512[:], tp[:16, :])
                for fb in range(8):
                    op = oppool.tile([P, P], fp, tag="op", name="op2")
                    for c in range(C):
                        nc.tensor.matmul(op[:], xts[c][:, fb, :],
                                         wt[:, fb, c, :],
                                         start=(c == 0), stop=(c == C - 1))
                    f0 = 64 * fb
                    if fb % 2 == 0:
                        nc.vector.tensor_copy(eo[:, f0:f0 + 64, :], op[:])
                    else:
                        nc.scalar.copy(eo[:, f0:f0 + 64, :], op[:])
                oc = ocpool.tile([P, 2], fp, tag="oc", name="oc2")
                nc.tensor.matmul(oc[:], x512[:], wt[:16, 8, 0, :2],
                                 start=True, stop=True)
                nc.vector.tensor_copy(eo[:, 512, :], oc[:])
                nc.scalar.dma_start(out_d[t0:t0 + P], eo[:])
    nc.compile()
    return nc


def _build_wt(bf):
    """bf: (F, C) complex64 -> Wt (128, FB, C, 128) bf16 (block-diag rot).

    fb<8: W[fb, c, (fl,ri), (fl2,ri2)] block-diagonal per fl.
    fb=8 slot [0:16, 8, 0, 0:2]: rows (c,ri), cols ri2 for f=512.
    """
    import ml_dtypes
    wa = bf.real.astype(np.float32)   # (F, C)
    wb = bf.imag.astype(np.float32)
    W = np.zeros((FB, C, P, P), np.float32)
    fl = np.arange(64)
    for fb in range(8):
        f = 64 * fb + fl
        for c in range(C):
            W[fb, c, 2 * fl, 2 * fl] = wa[f, c]
            W[fb, c, 2 * fl + 1, 2 * fl] = wb[f, c]
            W[fb, c, 2 * fl, 2 * fl + 1] = -wb[f, c]
            W[fb, c, 2 * fl + 1, 2 * fl + 1] = wa[f, c]
    for c in range(C):
        W[8, 0, 2 * c, 0] = wa[512, c]
        W[8, 0, 2 * c + 1, 0] = wb[512, c]
        W[8, 0, 2 * c, 1] = -wb[512, c]
        W[8, 0, 2 * c + 1, 1] = wa[512, c]
    return np.ascontiguousarray(
        W.transpose(2, 0, 1, 3)).astype(ml_dtypes.bfloat16)


def _decode_psd(raw, msum):
    """raw: (B,128,2560), msum: (B,128,10) -> psd_s, psd_n (B,F,C,C) c64."""
    nb = raw.shape[0]
    slots = np.empty((nb, 32, F, 16), np.float32)
    g = np.arange(F) // 4
    j = np.arange(F) % 4
    off = (g // 32) * BANK + (g % 32) * PSUM_W
    for jj in range(4):
        sel = j == jj
        cols = off[sel][:, None] + np.arange(16)[None]
        slots[:, :, sel, :] = raw[:, 32 * jj:32 * (jj + 1), :][
            :, :, cols.reshape(-1)].reshape(nb, 32, sel.sum(), 16)
    ms_sum = np.empty((nb, 2, F), np.float32)
    for mk in range(2):
        for ft, (f0, pf) in enumerate(F_TILES):
            ms_sum[:, mk, f0:f0 + pf] = msum[:, :pf, mk * 5 + ft]
    psds = []
    for mk in range(2):
        r = slots[:, 16 * mk:16 * mk + 16:2]     # (B, 8c, F, 16)
        i = slots[:, 16 * mk + 1:16 * mk + 16:2]
        re = r[..., 0::2] + i[..., 1::2]          # (B, c, F, e)
        im = i[..., 0::2] - r[..., 1::2]
        psd = (re + 1j * im).astype(np.complex64).transpose(0, 2, 1, 3)  # (B,F,c,e)
        scale = 1.0 / (ms_sum[:, mk] + C * EPS_PSD)
        psds.append(psd * scale[:, :, None, None].astype(np.complex64))
    return psds[0], psds[1]


def _middle(psd_s, psd_n, W_psd, b_psd, w_gvec, b_gvec):
    eye = np.eye(C, dtype=bool)
    psd = np.where(eye[None, None], np.complex64(0), psd_s)
    psd = np.swapaxes(psd.sum(axis=-1) / (C - 1), -1, -2)
    psd_feat = np.abs(psd).astype(np.float32)
    e = np.tanh(psd_feat @ W_psd + b_psd) @ w_gvec + b_gvec[0]
    e = SCALING * e
    e = e - e.max(axis=-1, keepdims=True)
    ex = np.exp(e)
    u = (ex / ex.sum(axis=-1, keepdims=True)).astype(np.float32)
    psd_n_reg = psd_n + (EPS_MVDR * np.eye(C)).astype(np.complex64)
    num = np.matmul(np.linalg.inv(psd_n_reg), psd_s)
    trace = np.einsum('bfcc->bf', num)
    ws = num / (trace[..., None, None] + EPS_MVDR)
    bf = np.einsum('bfec,bc->bfe', ws, u.astype(ws.dtype))  # (B,F,C)
    return bf


_RES = {"t_psd": None, "t_apply": None}


def _run(nc, in_maps, trace=False):
    from concourse.bass_utils import run_bass_kernel_spmd
    return run_bass_kernel_spmd(nc, in_maps, core_ids=list(range(B)),
                                trace=trace)


def _device_pipeline(data_ri, mask_speech, mask_noise,
                     W_psd, b_psd, w_gvec, b_gvec):
    # NTFF profiling corrupts execution on the non-profiled cores AND
    # poisons subsequent executions in the same process, so compute ALL
    # real outputs with clean runs first, then (if tracing was requested)
    # re-run both kernels traced purely for the timing numbers.
    trace = bool(os.environ.get("BEAM_TRACE"))
    nc1 = _build_psd_kernel()
    in1 = [{"data": data_ri[b],
            "ms": np.ascontiguousarray(mask_speech[b], np.float32),
            "mn": np.ascontiguousarray(mask_noise[b], np.float32)}
           for b in range(B)]
    r1 = _run(nc1, in1)
    raw = np.stack([r["psd_raw"] for r in r1.results])
    msum = np.stack([r["msum"] for r in r1.results])
    dbf = np.stack([r["data_bf"] for r in r1.results])

    psd_s, psd_n = _decode_psd(raw, msum)
    bf = _middle(psd_s, psd_n, W_psd, b_psd, w_gvec, b_gvec)

    nc2 = _build_apply_kernel_v2()
    in2 = [{"data_bf": dbf[b], "wt": _build_wt(bf[b])} for b in range(B)]
    r2 = _run(nc2, in2)
    out = np.stack([r["out"] for r in r2.results])

    if trace:
        _RES["t_psd"] = _run(nc1, in1, trace=True).exec_time_ns
        _RES["t_apply"] = _run(nc2, in2, trace=True).exec_time_ns
    return out


def _numpy_pipeline(data_ri, mask_speech, mask_noise,
                    W_psd, b_psd, w_gvec, b_gvec):
    data = data_ri[..., 0] + 1j * data_ri[..., 1]
    x = np.ascontiguousarray(np.transpose(data, (0, 3, 2, 1)))  # (B,F,C,T)
    psds = []
    for mask in (mask_speech, mask_noise):
        m = np.clip(mask, EPS_MASK, None).mean(axis=-2)
        m = m / (m.sum(axis=-1, keepdims=True) + EPS_PSD)
        xw = x * m[:, :, None, :].astype(x.dtype)
        psds.append(np.matmul(xw, np.conj(np.swapaxes(x, -1, -2))))
    bf = _middle(psds[0], psds[1], W_psd, b_psd, w_gvec, b_gvec)
    enh = np.einsum('bfc,bfct->bft', np.conj(bf), x)
    enh = np.swapaxes(enh, -1, -2)
    return np.stack([enh.real, enh.imag], axis=-1).astype(np.float32)


def kernel(data_ri, mask_speech, mask_noise, W_psd, b_psd, w_gvec, b_gvec,
           ilens):
    data_ri = np.ascontiguousarray(data_ri, dtype=np.float32)
    mask_speech = np.asarray(mask_speech, np.float32)
    mask_noise = np.asarray(mask_noise, np.float32)
    W_psd = np.asarray(W_psd, np.float32)
    b_psd = np.asarray(b_psd, np.float32)
    w_gvec = np.asarray(w_gvec, np.float32)
    b_gvec = np.asarray(b_gvec, np.float32)
    if os.environ.get("BEAM_NO_DEVICE"):
        return _numpy_pipeline(data_ri, mask_speech, mask_noise,
                               W_psd, b_psd, w_gvec, b_gvec)
    import signal
    old = None
    try:
        if hasattr(signal, "SIGALRM"):
            def _timeout(signum, frame):
                raise TimeoutError("device pipeline watchdog")
            old = signal.signal(signal.SIGALRM, _timeout)
            signal.alarm(900)
        return _device_pipeline(data_ri, mask_speech, mask_noise,
                                W_psd, b_psd, w_gvec, b_gvec)
    except Exception as exc:  # device unavailable -> still return correctly
        sys.stderr.write(f"device pipeline failed ({exc!r}); numpy fallback\n")
        return _numpy_pipeline(data_ri, mask_speech, mask_noise,
                               W_psd, b_psd, w_gvec, b_gvec)
    finally:
        if old is not None:
            signal.alarm(0)
            signal.signal(signal.SIGALRM, old)
